# revision 35
# baseline (speedup 1.0000x reference)
"""PASA group-softmax high-pass downsample kernel for 8 Trainium2 NeuronCores.

Reference computation (n=4, c=64, h=w=128, G=2 groups, K=3, stride 2):
  xp     = reflect_pad(x, 1)
  sigma  = conv3x3(xp, conv_w)                    # [n, 18, h, w]
  sigma  = sigma * bn_scale + bn_shift            # BN (inference)
  sigma  = softmax(sigma, axis=1)                 # over all 18 channels
  sigma  = onehot(center) - sigma                 # high-pass
  out[n,g,c,i,j] = sum_k patches[n,g,c,k,i,j] * sigma[n,g,k,i,j]
  return out[:, :, ::2, ::2]                      # [4, 64, 64, 64]

Device mapping (per core = one (image, 32-row half) shard):
  x is host-prepped into 2 parity planes [128, 33, 2, 66] bf16 with no
  duplication: partition p = 64*s + c holds channel c of sub-half s;
  plane row r, parity pl, col j = xp[c, 32s + r, 2j + pl].  Tap (dy, dx)
  maps to (pl, j0) = (dx & 1, dx >> 1); dx<2 taps are 4B-aligned (DVE 2x),
  the dx=2 taps are the PSUM-direct ones (1x regardless).

Schedule: chunk-pipelined over the two 8-row chunks (ch=0,1):
  conv ch  -> sigma[64ch:+64] as 18 (64x32)-tile matmuls (s=0/1 tiles
              concurrent), DMA-paced; exp ch (ACT); d ch (PE, 4-wide).
  apply ch -> per tap: one (64,128) broadcast esel_k @ e -> PSUM pair;
              ACT evacuates non-direct taps bf16; DVE mul patch * ebig
              (DIRECT taps straight from PSUM); (128,128) identity matmul
              accumulates into acc[ch].
  The softmax-denominator chain (recip, cast, rc broadcast+evac) and the
  chunk-0 tail (m = acc*rc on DVE, y = xc - m on GpSimd, store) thread
  into chunk 1's engine streams via callbacks; chunk-1 tail ends the run.
"""

import os
import ml_dtypes
import numpy as np

import concourse.bass as bass
import concourse.tile as tile
from concourse import bacc, mybir
from concourse.bass_utils import run_bass_kernel_spmd

F32 = mybir.dt.float32
BF16 = mybir.dt.bfloat16
FP8 = mybir.dt.float8e4

N, C, H, W = 4, 64, 128, 128
G, K = 2, 3
K2 = K * K
EPS = 1e-5
NCORES = 8
HO, WO = H // 2, W // 2            # 64 x 64 output spatial
ROWS_PER_CORE = HO // 2            # 32 output rows per core
ROWS_SUB = ROWS_PER_CORE // 2      # 16 output rows per sub-half (s=0,1)
PL_R = 2 * ROWS_SUB + 1            # 33 plane rows per sub-half
PL_W = WO + 2                      # 66 cols per parity plane (65 used)
POS_SUB = ROWS_SUB * WO            # 1024 positions per sub-half
CHUNK_ROWS = ROWS_SUB // 2         # 8 output rows per chunk
CHUNK = CHUNK_ROWS * WO            # 512 positions per chunk

# wpack tensor column layout (bf16): identity + conv weights + sel
OFF_POSI = 0
OFF_WTS = 128                      # [9, 32] flattened
OFF_SEL = OFF_WTS + K2 * 32        # 416
OFF_RSEL = OFF_SEL + 4             # 420: rselc [2, 128] (rows 0..3 used)
WPW = OFF_RSEL + 2 * 128           # 676
EPW = K2 * 128                     # esel pack [9, 128] flattened, fp8

NJUNK = 12                         # PE warm-up matmuls (512-col)
DIRECT = (2, 5, 8)                 # dx=2 taps: mul reads ebig from PSUM

_compiled = None


def _tap_slice(k, ch):
    """(row-unit offset, col offset) of tap k's patch in the plane view."""
    dy, dx = k // K, k % K
    u = 2 * (dy + 2 * CHUNK_ROWS * ch) + (dx & 1)
    return u, dx >> 1


def _build_program():
    nc = bacc.Bacc(
        "TRN2", target_bir_lowering=False, debug=False, num_devices=NCORES
    )

    xpl = nc.dram_tensor("xpl", [128, PL_R, 2, PL_W], BF16,
                         kind="ExternalInput")
    wpack = nc.dram_tensor("wpack", [128, WPW], BF16, kind="ExternalInput")
    epack = nc.dram_tensor("epack", [128, EPW], FP8, kind="ExternalInput")
    bias = nc.dram_tensor("bias", [128, 1], F32, kind="ExternalInput")
    y = nc.dram_tensor("y", [128, POS_SUB], BF16, kind="ExternalOutput")

    with tile.TileContext(nc) as tc:
        with (
            tc.tile_pool(name="singles", bufs=1) as singles,
            tc.tile_pool(name="psum", bufs=1, space="PSUM") as psum,
            tc.tile_pool(name="ebig", bufs=2, space="PSUM") as ebig_pool,
            tc.tile_pool(name="esb", bufs=3) as esb_pool,
            tc.tile_pool(name="prod", bufs=4) as prod_pool,
            tc.tile_pool(name="work", bufs=2) as work,
        ):
            # ---- input DMAs -------------------------------------------------
            # all three rings stream x, chunk-0 rows first, sized so each
            # ring carries a similar share; wpack rides first on sync
            # (small), esel slots mid-scalar (needed at apply ch0), rselc
            # last on gpsimd (needed at the tails).
            w_pk = singles.tile([128, WPW], BF16)
            nc.sync.dma_start(w_pk[:], wpack.ap())
            bias_sb = singles.tile([128, 1], F32)
            nc.scalar.dma_start(bias_sb[:], bias.ap())

            x_sb = singles.tile([128, PL_R, 2, PL_W], BF16)
            e_pk = singles.tile([128, EPW], FP8)
            r_mid = 2 * CHUNK_ROWS + 1
            nc.sync.dma_start(x_sb[0:44, 0:r_mid], xpl.ap()[0:44, 0:r_mid])
            nc.scalar.dma_start(x_sb[44:88, 0:r_mid],
                                xpl.ap()[44:88, 0:r_mid])
            nc.gpsimd.dma_start(x_sb[88:128, 0:r_mid],
                                xpl.ap()[88:128, 0:r_mid])
            nc.sync.dma_start(e_pk[:], epack.ap())
            nc.sync.dma_start(x_sb[44:88, r_mid:PL_R],
                              xpl.ap()[44:88, r_mid:PL_R])
            nc.scalar.dma_start(x_sb[0:44, r_mid:PL_R],
                                xpl.ap()[0:44, r_mid:PL_R])
            nc.gpsimd.dma_start(x_sb[88:128, r_mid:PL_R],
                                xpl.ap()[88:128, r_mid:PL_R])

            # prewarm ACT's exp table off the critical path
            warm_in = work.tile([1, 1], F32, tag="warm_in")
            nc.vector.memset(warm_in[:], 0.25)
            warm_e = work.tile([1, 1], F32, tag="warm_e")
            nc.scalar.activation(warm_e[:], warm_in[:],
                                 mybir.ActivationFunctionType.Exp)

            posI = w_pk[:, OFF_POSI : OFF_POSI + 128]
            w_sb = w_pk[:, OFF_WTS : OFF_WTS + K2 * 32].rearrange(
                "p (k o) -> p k o", k=K2
            )
            sel_sb = w_pk[:, OFF_SEL : OFF_SEL + 4]
            esel_sb = e_pk[:].rearrange("p (k o) -> p k o", k=K2)
            rc_sel = w_pk[0:4, OFF_RSEL : OFF_RSEL + 2 * 128].rearrange(
                "p (c o) -> p c o", c=2
            )
            # flattened 3D view of the planes: row-unit = 2*r + parity
            xv = x_sb[:].rearrange("p r d c -> p (r d) c")

            # ---- PE warm-up: junk (64x32)-tile matmuls on memset weights
            # (no DMA dependency) keep the HAM clock warm while x streams
            # in; they write (and are cleared from) the acc0 bank.
            junk_w = singles.tile([64, CHUNK], BF16)
            nc.vector.memset(junk_w[:], 0.5)
            sigma_ps = psum.tile([128, CHUNK], F32, tag="sigma")
            d_ps = psum.tile([4, CHUNK], F32, tag="dr")
            acc_ps = psum.tile([128, 2 * CHUNK], F32, tag="acc")
            for i in range(NJUNK):
                nc.tensor.matmul(acc_ps[0:32, 0:CHUNK], junk_w[:, 0:32],
                                 junk_w[:],
                                 start=(i == 0), stop=(i == NJUNK - 1),
                                 tile_position=(0, 0),
                                 skip_group_check=True)

            e_t = [singles.tile([128, CHUNK], BF16, name=f"e{c}", tag=f"e{c}")
                   for c in (0, 1)]

            def conv_chunk(ch):
                # 18 matmuls, 2-way tile-concurrent (s=0 and s=1 land on
                # disjoint (row, col) PE tiles); sigma q-group = s + 2*ch.
                for k in range(K2):
                    u, j0 = _tap_slice(k, ch)
                    for s in range(2):
                        q = s + 2 * ch
                        nc.tensor.matmul(
                            sigma_ps[32 * q : 32 * q + 32, :],
                            w_sb[64 * s : 64 * s + 64, k, :],
                            xv[64 * s : 64 * s + 64,
                               u : u + 4 * CHUNK_ROWS - 3 : 4,
                               j0 : j0 + WO],
                            start=(k == 0), stop=(k == K2 - 1),
                            tile_position=(64 * s, 32 * q),
                            skip_group_check=True,
                        )
                # exp (ACT); the d matmuls are emitted after both conv
                # chunks so they never stall the PE queue on an exp.
                p0 = 64 * ch
                nc.scalar.activation(
                    e_t[ch][p0 : p0 + 64, :], sigma_ps[p0 : p0 + 64, :],
                    mybir.ActivationFunctionType.Exp,
                    bias=bias_sb[p0 : p0 + 64, :], scale=1.0,
                )

            def d_chunk(ch):
                # 4-wide output (2-partition outputs fail BIR verification);
                # the off-chunk sel columns are zero, so chunk 1 accumulates
                # its two d rows without disturbing chunk 0's.
                p0 = 64 * ch
                nc.tensor.matmul(
                    d_ps[0:4, :],
                    sel_sb[p0 : p0 + 64, 0:4],
                    e_t[ch][p0 : p0 + 64, :],
                    start=(ch == 0), stop=(ch == 1),
                    tile_position=(p0, 0), skip_group_check=True,
                )

            r_sb = singles.tile([4, CHUNK], F32)
            r_bf = singles.tile([4, CHUNK], BF16)
            rc_sb = [singles.tile([128, CHUNK], BF16, name=f"rcsb{c}",
                                  tag=f"rcsb{c}") for c in (0, 1)]
            y_sb = work.tile([128, POS_SUB], BF16, tag="ysb")

            def recip_cb():
                nc.vector.reciprocal_approx_fast(r_sb[:], d_ps[:])
                nc.vector.tensor_copy(r_bf[:], r_sb[:])

            def rc_cb():
                for ch in range(2):
                    t = psum.tile([128, CHUNK], F32, name=f"rc{ch}",
                                  tag="sigma" if ch == 0 else "dr")
                    nc.tensor.matmul(t[:], rc_sel[:, ch, :], r_bf[:],
                                     tile_position=(0, 0),
                                     skip_group_check=True)
                    nc.scalar.copy(rc_sb[ch][:], t[:])

            def apply_all(cbs):
                # per tap: 2 row-tile-paired (64,128) broadcasts (one per
                # chunk, concurrent on disjoint PE row halves) -> [128,1024]
                # PSUM pair; ACT evacuates non-DIRECT taps bf16; one
                # full-width DVE mul; 2 (128,128) identity matmuls
                # accumulate into acc.  cbs threads the denominator chain
                # into the apply streams.
                # process the PSUM-direct taps at the head of each trio:
                # their short ebig->mul chain fills the pipeline while the
                # first evacuations are still in flight.
                order = (2, 0, 1, 5, 3, 4, 8, 6, 7)
                for i, k in enumerate(order):
                    eb = ebig_pool.tile([128, 2 * CHUNK], F32,
                                        name=f"ebig{k}", tag="ebig")
                    for ch in range(2):
                        nc.tensor.matmul(
                            eb[:, CHUNK * ch : CHUNK * (ch + 1)],
                            esel_sb[64 * ch : 64 * ch + 64, k, :],
                            e_t[ch][64 * ch : 64 * ch + 64, :],
                            tile_position=(64 * ch, 0),
                            skip_group_check=True,
                        )
                    u, j0 = _tap_slice(k, 0)
                    patch = xv[:, u : u + 4 * ROWS_SUB - 3 : 4, j0 : j0 + WO]
                    prod = prod_pool.tile(
                        [128, ROWS_SUB, WO], BF16,
                        name=f"prod{k}", tag="prod",
                    )
                    ebv = eb[:].rearrange("p (r c) -> p r c", r=ROWS_SUB)
                    if k in DIRECT:
                        nc.vector.tensor_mul(prod[:], patch, ebv)
                    else:
                        sb = esb_pool.tile([128, ROWS_SUB, WO], BF16,
                                           name=f"esb{k}", tag="esb")
                        nc.scalar.copy(sb[:], ebv)
                        nc.vector.tensor_mul(prod[:], patch, sb[:])
                    pf = prod[:].rearrange("p r c -> p (r c)")
                    for ch in range(2):
                        nc.tensor.matmul(
                            acc_ps[:, CHUNK * ch : CHUNK * (ch + 1)],
                            posI,
                            pf[:, CHUNK * ch : CHUNK * (ch + 1)],
                            start=(i == 0), stop=(i == K2 - 1),
                            tile_position=(0, 0),
                            skip_group_check=True,
                        )
                    if i in cbs:
                        cbs[i]()

            def tail_chunk(ch):
                cs = slice(CHUNK * ch, CHUNK * (ch + 1))
                m_sb = work.tile([128, CHUNK_ROWS, WO], BF16, tag=f"m{ch}")
                nc.vector.tensor_mul(
                    m_sb[:],
                    acc_ps[:, cs].rearrange("p (r c) -> p r c", r=CHUNK_ROWS),
                    rc_sb[ch][:].rearrange("p (r c) -> p r c", r=CHUNK_ROWS),
                )
                u, j0 = _tap_slice(4, ch)  # center tap (dy=1, dx=1)
                xc = xv[:, u : u + 4 * CHUNK_ROWS - 3 : 4, j0 : j0 + WO]
                # chunk 0's sub rides the idle GpSimd engine; chunk 1's is
                # on the critical tail, DVE (2x bf16).
                eng = nc.gpsimd if ch == 0 else nc.vector
                eng.tensor_sub(
                    y_sb[:, cs].rearrange("p (r c) -> p r c", r=CHUNK_ROWS),
                    xc, m_sb[:],
                )
                eng = nc.sync if ch == 0 else nc.scalar
                eng.dma_start(y.ap()[:, cs], y_sb[:, cs])

            conv_chunk(0)
            conv_chunk(1)
            d_chunk(0)
            d_chunk(1)
            apply_all(cbs={1: recip_cb, 3: rc_cb})
            tail_chunk(0)
            tail_chunk(1)

    nc.compile()
    return nc


def _host_inputs(x, conv_w, gamma, beta, running_mean, running_var):
    """Per-core input dicts: BN folding + reflect pad + parity-plane layout."""
    scale = gamma / np.sqrt(running_var + EPS)
    shift = beta - running_mean * scale

    # conv weights as lhsT [tap, c, o] * bn_scale, padded to 32 outs, dup'd
    w_scaled = conv_w * scale[:, None, None, None]           # [18, 64, 3, 3]
    wl = np.transpose(w_scaled, (2, 3, 1, 0)).reshape(K2, C, G * K2)
    wl32 = np.zeros((K2, C, 32), np.float32)
    wl32[:, :, : G * K2] = wl
    wts = np.ascontiguousarray(
        np.concatenate([wl32, wl32], axis=1).transpose(1, 0, 2)
    ).reshape(128, K2 * 32)

    wpk = np.zeros((128, WPW), np.float32)
    wpk[:, OFF_POSI : OFF_POSI + 128] = np.eye(128)
    wpk[:, OFF_WTS : OFF_WTS + K2 * 32] = wts
    for q in range(4):
        wpk[32 * q : 32 * q + G * K2, OFF_SEL + q] = 1.0           # sel
    for ch in range(2):
        for p in range(128):
            wpk[p // 64 + 2 * ch, OFF_RSEL + 128 * ch + p] = 1.0   # rselc
    esel = np.zeros((128, K2, 128), np.float32)
    for k in range(K2):
        for p in range(128):
            s, g = p // 64, (p % 64) // 32
            for ch in range(2):
                esel[32 * (s + 2 * ch) + K2 * g + k, k, p] = 1.0
    epk = esel.reshape(128, K2 * 128)
    wpk = wpk.astype(ml_dtypes.bfloat16)
    epk = epk.astype(ml_dtypes.float8_e4m3)

    bias = np.zeros((128, 1), np.float32)
    for q in range(4):
        bias[32 * q : 32 * q + G * K2, 0] = shift

    xpad = np.pad(x, ((0, 0), (0, 0), (1, 1), (1, 1)), mode="reflect")

    in_maps = []
    for core in range(NCORES):
        n, h = core // 2, core % 2
        r0 = 64 * h
        xpl = np.zeros((128, PL_R, 2, PL_W), np.float32)
        for s in range(2):
            sl = xpad[n, :, r0 + 32 * s : r0 + 32 * s + PL_R, :]  # [64,33,130]
            xpl[64 * s : 64 * s + 64, :, 0, 0:65] = sl[:, :, 0::2]
            xpl[64 * s : 64 * s + 64, :, 1, 0:65] = sl[:, :, 1::2]
        in_maps.append(
            {"xpl": xpl.astype(ml_dtypes.bfloat16), "wpack": wpk,
             "epack": epk, "bias": bias}
        )
    return in_maps


def _gather_output(results):
    out = np.empty((N, C, HO, WO), np.float32)
    for core, res in enumerate(results):
        n, h = core // 2, core % 2
        ycore = np.asarray(res["y"]).astype(np.float32).reshape(
            2, C, ROWS_SUB, WO
        )
        out[n, :, 32 * h : 32 * h + ROWS_SUB, :] = ycore[0]
        out[n, :, 32 * h + ROWS_SUB : 32 * h + 2 * ROWS_SUB, :] = ycore[1]
    return out


def _ensure_ntff_hook():
    """Install the axon NTFF profile hook if the image's antenv lacks it."""
    try:
        from antenv import axon_hooks  # noqa: F401
        return
    except ImportError:
        pass
    try:
        import sys
        import types

        import antenv
        from trn_agent_boot.trn_boot import _ntff_profile_via_ctypes

        hook = _ntff_profile_via_ctypes("/opt/axon/libaxon_pjrt.so")
        mod = types.ModuleType("antenv.axon_hooks")
        state = {"hook": hook}
        mod.get_axon_ntff_profile_hook = lambda: state["hook"]
        mod.set_axon_ntff_profile_hook = lambda h: state.update(hook=h)
        sys.modules["antenv.axon_hooks"] = mod
        antenv.axon_hooks = mod
    except Exception:
        pass


def kernel(x, conv_w, gamma, beta, running_mean, running_var):
    global _compiled
    x = np.asarray(x, np.float32)
    conv_w = np.asarray(conv_w, np.float32)
    gamma = np.asarray(gamma, np.float32)
    beta = np.asarray(beta, np.float32)
    running_mean = np.asarray(running_mean, np.float32)
    running_var = np.asarray(running_var, np.float32)

    if _compiled is None:
        _compiled = _build_program()
    nc = _compiled

    in_maps = _host_inputs(x, conv_w, gamma, beta, running_mean, running_var)
    trace = bool(int(os.environ.get("PASA_TRACE", "0")))
    if trace:
        _ensure_ntff_hook()
    res = run_bass_kernel_spmd(
        nc, in_maps, core_ids=list(range(NCORES)), trace=trace
    )
    kernel.last_results = res
    return _gather_output(res.results)


if __name__ == "__main__":
    # quick CoreSim check of core 0 against a numpy re-implementation
    from concourse.bass_interp import CoreSim

    rng = np.random.default_rng(0)
    x = rng.standard_normal((N, C, H, W)).astype(np.float32)
    conv_w = (rng.standard_normal((G * K2, C, K, K)).astype(np.float32)
              * np.sqrt(2.0 / (G * K2 * K * K)))
    gamma = rng.uniform(0.5, 1.5, G * K2).astype(np.float32)
    beta = (rng.standard_normal(G * K2) * 0.1).astype(np.float32)
    rmean = (rng.standard_normal(G * K2) * 0.1).astype(np.float32)
    rvar = rng.uniform(0.5, 1.5, G * K2).astype(np.float32)

    nc = _build_program()
    in_maps = _host_inputs(x, conv_w, gamma, beta, rmean, rvar)
    sim = CoreSim(nc)
    for kk, v in in_maps[0].items():
        sim.tensor(kk)[:] = v
    sim.simulate(check_with_hw=False)
    ysim = np.asarray(sim.tensor("y")).astype(np.float32).reshape(
        2, C, ROWS_SUB, WO
    )

    # numpy reference for core 0 region (image 0, output rows 0..32)
    scale = gamma / np.sqrt(rvar + EPS)
    shift = beta - rmean * scale
    xpad = np.pad(x[0], ((0, 0), (1, 1), (1, 1)), mode="reflect")
    sig = np.zeros((G * K2, 32, WO), np.float32)
    for o in range(G * K2):
        for dy in range(K):
            for dx in range(K):
                sig[o] += np.einsum(
                    "crw->rw",
                    conv_w[o, :, dy, dx][:, None, None]
                    * xpad[:, dy : dy + 64 : 2, dx : dx + 128 : 2],
                )
    sig = sig * scale[:, None, None] + shift[:, None, None]
    e = np.exp(sig)
    r = 1.0 / e.sum(0)
    acc = np.zeros((C, 32, WO), np.float32)
    for g in range(G):
        for k in range(K2):
            dy, dx = k // K, k % K
            acc[32 * g : 32 * g + 32] += (
                xpad[32 * g : 32 * g + 32, dy : dy + 64 : 2, dx : dx + 128 : 2]
                * e[g * K2 + k][None]
            )
    ref = (xpad[:, 1:65:2, 1:129:2] - acc * r[None]).astype(np.float32)

    got = np.concatenate([ysim[0], ysim[1]], axis=1)
    err = np.abs(got - ref).max() / np.abs(ref).max()
    print("sim rel err:", err)


# revision 36
# speedup vs baseline: 1.0096x; 1.0096x over previous
"""PASA group-softmax high-pass downsample kernel for 8 Trainium2 NeuronCores.

Reference computation (n=4, c=64, h=w=128, G=2 groups, K=3, stride 2):
  xp     = reflect_pad(x, 1)
  sigma  = conv3x3(xp, conv_w)                    # [n, 18, h, w]
  sigma  = sigma * bn_scale + bn_shift            # BN (inference)
  sigma  = softmax(sigma, axis=1)                 # over all 18 channels
  sigma  = onehot(center) - sigma                 # high-pass
  out[n,g,c,i,j] = sum_k patches[n,g,c,k,i,j] * sigma[n,g,k,i,j]
  return out[:, :, ::2, ::2]                      # [4, 64, 64, 64]

Device mapping (per core = one (image, 32-row half) shard):
  x is host-prepped into 2 parity planes [128, 33, 2, 66] bf16 with no
  duplication: partition p = 64*s + c holds channel c of sub-half s;
  plane row r, parity pl, col j = xp[c, 32s + r, 2j + pl].  Tap (dy, dx)
  maps to (pl, j0) = (dx & 1, dx >> 1); dx<2 taps are 4B-aligned (DVE 2x),
  the dx=2 taps are the PSUM-direct ones (1x regardless).

Schedule: chunk-pipelined over the two 8-row chunks (ch=0,1):
  conv ch  -> sigma[64ch:+64] as 18 (64x32)-tile matmuls (s=0/1 tiles
              concurrent), DMA-paced; exp ch (ACT); d ch (PE, 4-wide).
  apply ch -> per tap: one (64,128) broadcast esel_k @ e -> PSUM pair;
              ACT evacuates non-direct taps bf16; DVE mul patch * ebig
              (DIRECT taps straight from PSUM); (128,128) identity matmul
              accumulates into acc[ch].
  The softmax-denominator chain (recip, cast, rc broadcast+evac) and the
  chunk-0 tail (m = acc*rc on DVE, y = xc - m on GpSimd, store) thread
  into chunk 1's engine streams via callbacks; chunk-1 tail ends the run.
"""

import os
import ml_dtypes
import numpy as np

import concourse.bass as bass
import concourse.tile as tile
from concourse import bacc, mybir
from concourse.bass_utils import run_bass_kernel_spmd

F32 = mybir.dt.float32
BF16 = mybir.dt.bfloat16
FP8 = mybir.dt.float8e4

N, C, H, W = 4, 64, 128, 128
G, K = 2, 3
K2 = K * K
EPS = 1e-5
NCORES = 8
HO, WO = H // 2, W // 2            # 64 x 64 output spatial
ROWS_PER_CORE = HO // 2            # 32 output rows per core
ROWS_SUB = ROWS_PER_CORE // 2      # 16 output rows per sub-half (s=0,1)
PL_R = 2 * ROWS_SUB + 1            # 33 plane rows per sub-half
PL_W = WO + 2                      # 66 cols per parity plane (65 used)
POS_SUB = ROWS_SUB * WO            # 1024 positions per sub-half
CHUNK_ROWS = ROWS_SUB // 2         # 8 output rows per chunk
CHUNK = CHUNK_ROWS * WO            # 512 positions per chunk

# wpack tensor column layout (bf16): identity + conv weights + sel
OFF_POSI = 0
OFF_WTS = 128                      # [9, 32] flattened
OFF_SEL = OFF_WTS + K2 * 32        # 416
OFF_RSEL = OFF_SEL + 4             # 420: rselc [2, 128] (rows 0..3 used)
WPW = OFF_RSEL + 2 * 128           # 676
EPW = K2 * 128                     # esel pack [9, 128] flattened, fp8

NJUNK = 12                         # PE warm-up matmuls (512-col)
DIRECT = (2, 5, 8)                 # dx=2 taps: mul reads ebig from PSUM

_compiled = None


def _tap_slice(k, ch):
    """(row-unit offset, col offset) of tap k's patch in the plane view."""
    dy, dx = k // K, k % K
    u = 2 * (dy + 2 * CHUNK_ROWS * ch) + (dx & 1)
    return u, dx >> 1


def _build_program():
    nc = bacc.Bacc(
        "TRN2", target_bir_lowering=False, debug=False, num_devices=NCORES
    )

    xpl = nc.dram_tensor("xpl", [128, PL_R, 2, PL_W], BF16,
                         kind="ExternalInput")
    wpack = nc.dram_tensor("wpack", [128, WPW], BF16, kind="ExternalInput")
    epack = nc.dram_tensor("epack", [128, EPW], FP8, kind="ExternalInput")
    bias = nc.dram_tensor("bias", [128, 1], F32, kind="ExternalInput")
    y = nc.dram_tensor("y", [128, POS_SUB], BF16, kind="ExternalOutput")

    with tile.TileContext(nc) as tc:
        with (
            tc.tile_pool(name="singles", bufs=1) as singles,
            tc.tile_pool(name="psum", bufs=1, space="PSUM") as psum,
            tc.tile_pool(name="ebig", bufs=2, space="PSUM") as ebig_pool,
            tc.tile_pool(name="esb", bufs=3) as esb_pool,
            tc.tile_pool(name="prod", bufs=4) as prod_pool,
            tc.tile_pool(name="work", bufs=2) as work,
        ):
            # ---- input DMAs -------------------------------------------------
            # all three rings stream x, chunk-0 rows first, sized so each
            # ring carries a similar share; wpack rides first on sync
            # (small), esel slots mid-scalar (needed at apply ch0), rselc
            # last on gpsimd (needed at the tails).
            w_pk = singles.tile([128, WPW], BF16)
            nc.sync.dma_start(w_pk[:], wpack.ap())
            bias_sb = singles.tile([128, 1], F32)
            nc.scalar.dma_start(bias_sb[:], bias.ap())

            x_sb = singles.tile([128, PL_R, 2, PL_W], BF16)
            e_pk = singles.tile([128, EPW], FP8)
            r_mid = 2 * CHUNK_ROWS + 1
            nc.sync.dma_start(x_sb[0:44, 0:r_mid], xpl.ap()[0:44, 0:r_mid])
            nc.scalar.dma_start(x_sb[44:88, 0:r_mid],
                                xpl.ap()[44:88, 0:r_mid])
            nc.gpsimd.dma_start(x_sb[88:128, 0:r_mid],
                                xpl.ap()[88:128, 0:r_mid])
            nc.sync.dma_start(e_pk[:], epack.ap())
            nc.sync.dma_start(x_sb[44:88, r_mid:PL_R],
                              xpl.ap()[44:88, r_mid:PL_R])
            nc.scalar.dma_start(x_sb[0:44, r_mid:PL_R],
                                xpl.ap()[0:44, r_mid:PL_R])
            nc.gpsimd.dma_start(x_sb[88:128, r_mid:PL_R],
                                xpl.ap()[88:128, r_mid:PL_R])

            # prewarm ACT's exp table off the critical path
            warm_in = work.tile([1, 1], F32, tag="warm_in")
            nc.vector.memset(warm_in[:], 0.25)
            warm_e = work.tile([1, 1], F32, tag="warm_e")
            nc.scalar.activation(warm_e[:], warm_in[:],
                                 mybir.ActivationFunctionType.Exp)

            posI = w_pk[:, OFF_POSI : OFF_POSI + 128]
            w_sb = w_pk[:, OFF_WTS : OFF_WTS + K2 * 32].rearrange(
                "p (k o) -> p k o", k=K2
            )
            sel_sb = w_pk[:, OFF_SEL : OFF_SEL + 4]
            esel_sb = e_pk[:].rearrange("p (k o) -> p k o", k=K2)
            rc_sel = w_pk[0:4, OFF_RSEL : OFF_RSEL + 2 * 128].rearrange(
                "p (c o) -> p c o", c=2
            )
            # flattened 3D view of the planes: row-unit = 2*r + parity
            xv = x_sb[:].rearrange("p r d c -> p (r d) c")

            # ---- PE warm-up: junk (64x32)-tile matmuls on memset weights
            # (no DMA dependency) keep the HAM clock warm while x streams
            # in; they write (and are cleared from) the acc0 bank.
            junk_w = singles.tile([64, CHUNK], BF16)
            nc.vector.memset(junk_w[:], 0.5)
            sigma_ps = psum.tile([128, CHUNK], F32, tag="sigma")
            d_ps = psum.tile([4, CHUNK], F32, tag="dr")
            acc_ps = psum.tile([128, 2 * CHUNK], F32, tag="acc")
            for i in range(NJUNK):
                nc.tensor.matmul(acc_ps[0:32, 0:CHUNK], junk_w[:, 0:32],
                                 junk_w[:],
                                 start=(i == 0), stop=(i == NJUNK - 1),
                                 tile_position=(0, 0),
                                 skip_group_check=True)

            e_t = [singles.tile([128, CHUNK], BF16, name=f"e{c}", tag=f"e{c}")
                   for c in (0, 1)]

            def conv_chunk(ch):
                # 18 matmuls, 2-way tile-concurrent (s=0 and s=1 land on
                # disjoint (row, col) PE tiles); sigma q-group = s + 2*ch.
                for k in range(K2):
                    u, j0 = _tap_slice(k, ch)
                    for s in range(2):
                        q = s + 2 * ch
                        nc.tensor.matmul(
                            sigma_ps[32 * q : 32 * q + 32, :],
                            w_sb[64 * s : 64 * s + 64, k, :],
                            xv[64 * s : 64 * s + 64,
                               u : u + 4 * CHUNK_ROWS - 3 : 4,
                               j0 : j0 + WO],
                            start=(k == 0), stop=(k == K2 - 1),
                            tile_position=(64 * s, 32 * q),
                            skip_group_check=True,
                        )
                # exp (ACT); the d matmuls are emitted after both conv
                # chunks so they never stall the PE queue on an exp.
                p0 = 64 * ch
                nc.scalar.activation(
                    e_t[ch][p0 : p0 + 64, :], sigma_ps[p0 : p0 + 64, :],
                    mybir.ActivationFunctionType.Exp,
                    bias=bias_sb[p0 : p0 + 64, :], scale=1.0,
                )

            def d_chunk(ch):
                # 4-wide output (2-partition outputs fail BIR verification);
                # the off-chunk sel columns are zero, so chunk 1 accumulates
                # its two d rows without disturbing chunk 0's.
                p0 = 64 * ch
                nc.tensor.matmul(
                    d_ps[0:4, :],
                    sel_sb[p0 : p0 + 64, 0:4],
                    e_t[ch][p0 : p0 + 64, :],
                    start=(ch == 0), stop=(ch == 1),
                    tile_position=(p0, 0), skip_group_check=True,
                )

            r_sb = singles.tile([4, CHUNK], F32)
            r_bf = singles.tile([4, CHUNK], BF16)
            rc_sb = [singles.tile([128, CHUNK], BF16, name=f"rcsb{c}",
                                  tag=f"rcsb{c}") for c in (0, 1)]
            y_sb = work.tile([128, POS_SUB], BF16, tag="ysb")

            def recip_cb():
                nc.vector.reciprocal_approx_fast(r_sb[:], d_ps[:])
                nc.vector.tensor_copy(r_bf[:], r_sb[:])

            def rc_cb():
                for ch in range(2):
                    t = psum.tile([128, CHUNK], F32, name=f"rc{ch}",
                                  tag="sigma" if ch == 0 else "dr")
                    nc.tensor.matmul(t[:], rc_sel[:, ch, :], r_bf[:],
                                     tile_position=(0, 0),
                                     skip_group_check=True)
                    nc.scalar.copy(rc_sb[ch][:], t[:])

            def apply_all(cbs):
                # per tap: 2 row-tile-paired (64,128) broadcasts (one per
                # chunk, concurrent on disjoint PE row halves) -> [128,1024]
                # PSUM pair; ACT evacuates non-DIRECT taps bf16; one
                # full-width DVE mul; 2 (128,128) identity matmuls
                # accumulate into acc.  cbs threads the denominator chain
                # into the apply streams.
                for k in range(K2):
                    eb = ebig_pool.tile([128, 2 * CHUNK], F32,
                                        name=f"ebig{k}", tag="ebig")
                    for ch in range(2):
                        nc.tensor.matmul(
                            eb[:, CHUNK * ch : CHUNK * (ch + 1)],
                            esel_sb[64 * ch : 64 * ch + 64, k, :],
                            e_t[ch][64 * ch : 64 * ch + 64, :],
                            tile_position=(64 * ch, 0),
                            skip_group_check=True,
                        )
                    u, j0 = _tap_slice(k, 0)
                    patch = xv[:, u : u + 4 * ROWS_SUB - 3 : 4, j0 : j0 + WO]
                    prod = prod_pool.tile(
                        [128, ROWS_SUB, WO], BF16,
                        name=f"prod{k}", tag="prod",
                    )
                    ebv = eb[:].rearrange("p (r c) -> p r c", r=ROWS_SUB)
                    if k in DIRECT:
                        nc.vector.tensor_mul(prod[:], patch, ebv)
                    else:
                        sb = esb_pool.tile([128, ROWS_SUB, WO], BF16,
                                           name=f"esb{k}", tag="esb")
                        nc.scalar.copy(sb[:], ebv)
                        nc.vector.tensor_mul(prod[:], patch, sb[:])
                    pf = prod[:].rearrange("p r c -> p (r c)")
                    for ch in range(2):
                        nc.tensor.matmul(
                            acc_ps[:, CHUNK * ch : CHUNK * (ch + 1)],
                            posI,
                            pf[:, CHUNK * ch : CHUNK * (ch + 1)],
                            start=(k == 0), stop=(k == K2 - 1),
                            tile_position=(0, 0),
                            skip_group_check=True,
                        )
                    if k in cbs:
                        cbs[k]()

            def tail_chunk(ch):
                cs = slice(CHUNK * ch, CHUNK * (ch + 1))
                m_sb = work.tile([128, CHUNK_ROWS, WO], BF16, tag=f"m{ch}")
                nc.vector.tensor_mul(
                    m_sb[:],
                    acc_ps[:, cs].rearrange("p (r c) -> p r c", r=CHUNK_ROWS),
                    rc_sb[ch][:].rearrange("p (r c) -> p r c", r=CHUNK_ROWS),
                )
                u, j0 = _tap_slice(4, ch)  # center tap (dy=1, dx=1)
                xc = xv[:, u : u + 4 * CHUNK_ROWS - 3 : 4, j0 : j0 + WO]
                # chunk 0's sub rides the idle GpSimd engine; chunk 1's is
                # on the critical tail, DVE (2x bf16).
                eng = nc.gpsimd if ch == 0 else nc.vector
                eng.tensor_sub(
                    y_sb[:, cs].rearrange("p (r c) -> p r c", r=CHUNK_ROWS),
                    xc, m_sb[:],
                )
                eng = nc.sync if ch == 0 else nc.scalar
                eng.dma_start(y.ap()[:, cs], y_sb[:, cs])

            conv_chunk(0)
            conv_chunk(1)
            d_chunk(0)
            d_chunk(1)
            apply_all(cbs={1: recip_cb, 3: rc_cb})
            tail_chunk(0)
            tail_chunk(1)

    nc.compile()
    return nc


def _host_inputs(x, conv_w, gamma, beta, running_mean, running_var):
    """Per-core input dicts: BN folding + reflect pad + parity-plane layout."""
    scale = gamma / np.sqrt(running_var + EPS)
    shift = beta - running_mean * scale

    # conv weights as lhsT [tap, c, o] * bn_scale, padded to 32 outs, dup'd
    w_scaled = conv_w * scale[:, None, None, None]           # [18, 64, 3, 3]
    wl = np.transpose(w_scaled, (2, 3, 1, 0)).reshape(K2, C, G * K2)
    wl32 = np.zeros((K2, C, 32), np.float32)
    wl32[:, :, : G * K2] = wl
    wts = np.ascontiguousarray(
        np.concatenate([wl32, wl32], axis=1).transpose(1, 0, 2)
    ).reshape(128, K2 * 32)

    wpk = np.zeros((128, WPW), np.float32)
    wpk[:, OFF_POSI : OFF_POSI + 128] = np.eye(128)
    wpk[:, OFF_WTS : OFF_WTS + K2 * 32] = wts
    for q in range(4):
        wpk[32 * q : 32 * q + G * K2, OFF_SEL + q] = 1.0           # sel
    for ch in range(2):
        for p in range(128):
            wpk[p // 64 + 2 * ch, OFF_RSEL + 128 * ch + p] = 1.0   # rselc
    esel = np.zeros((128, K2, 128), np.float32)
    for k in range(K2):
        for p in range(128):
            s, g = p // 64, (p % 64) // 32
            for ch in range(2):
                esel[32 * (s + 2 * ch) + K2 * g + k, k, p] = 1.0
    epk = esel.reshape(128, K2 * 128)
    wpk = wpk.astype(ml_dtypes.bfloat16)
    epk = epk.astype(ml_dtypes.float8_e4m3)

    bias = np.zeros((128, 1), np.float32)
    for q in range(4):
        bias[32 * q : 32 * q + G * K2, 0] = shift

    xpad = np.pad(x, ((0, 0), (0, 0), (1, 1), (1, 1)), mode="reflect")

    in_maps = []
    for core in range(NCORES):
        n, h = core // 2, core % 2
        r0 = 64 * h
        xpl = np.zeros((128, PL_R, 2, PL_W), np.float32)
        for s in range(2):
            sl = xpad[n, :, r0 + 32 * s : r0 + 32 * s + PL_R, :]  # [64,33,130]
            xpl[64 * s : 64 * s + 64, :, 0, 0:65] = sl[:, :, 0::2]
            xpl[64 * s : 64 * s + 64, :, 1, 0:65] = sl[:, :, 1::2]
        in_maps.append(
            {"xpl": xpl.astype(ml_dtypes.bfloat16), "wpack": wpk,
             "epack": epk, "bias": bias}
        )
    return in_maps


def _gather_output(results):
    out = np.empty((N, C, HO, WO), np.float32)
    for core, res in enumerate(results):
        n, h = core // 2, core % 2
        ycore = np.asarray(res["y"]).astype(np.float32).reshape(
            2, C, ROWS_SUB, WO
        )
        out[n, :, 32 * h : 32 * h + ROWS_SUB, :] = ycore[0]
        out[n, :, 32 * h + ROWS_SUB : 32 * h + 2 * ROWS_SUB, :] = ycore[1]
    return out


def _ensure_ntff_hook():
    """Install the axon NTFF profile hook if the image's antenv lacks it."""
    try:
        from antenv import axon_hooks  # noqa: F401
        return
    except ImportError:
        pass
    try:
        import sys
        import types

        import antenv
        from trn_agent_boot.trn_boot import _ntff_profile_via_ctypes

        hook = _ntff_profile_via_ctypes("/opt/axon/libaxon_pjrt.so")
        mod = types.ModuleType("antenv.axon_hooks")
        state = {"hook": hook}
        mod.get_axon_ntff_profile_hook = lambda: state["hook"]
        mod.set_axon_ntff_profile_hook = lambda h: state.update(hook=h)
        sys.modules["antenv.axon_hooks"] = mod
        antenv.axon_hooks = mod
    except Exception:
        pass


def kernel(x, conv_w, gamma, beta, running_mean, running_var):
    global _compiled
    x = np.asarray(x, np.float32)
    conv_w = np.asarray(conv_w, np.float32)
    gamma = np.asarray(gamma, np.float32)
    beta = np.asarray(beta, np.float32)
    running_mean = np.asarray(running_mean, np.float32)
    running_var = np.asarray(running_var, np.float32)

    if _compiled is None:
        _compiled = _build_program()
    nc = _compiled

    in_maps = _host_inputs(x, conv_w, gamma, beta, running_mean, running_var)
    trace = bool(int(os.environ.get("PASA_TRACE", "0")))
    if trace:
        _ensure_ntff_hook()
    res = run_bass_kernel_spmd(
        nc, in_maps, core_ids=list(range(NCORES)), trace=trace
    )
    kernel.last_results = res
    return _gather_output(res.results)


if __name__ == "__main__":
    # quick CoreSim check of core 0 against a numpy re-implementation
    from concourse.bass_interp import CoreSim

    rng = np.random.default_rng(0)
    x = rng.standard_normal((N, C, H, W)).astype(np.float32)
    conv_w = (rng.standard_normal((G * K2, C, K, K)).astype(np.float32)
              * np.sqrt(2.0 / (G * K2 * K * K)))
    gamma = rng.uniform(0.5, 1.5, G * K2).astype(np.float32)
    beta = (rng.standard_normal(G * K2) * 0.1).astype(np.float32)
    rmean = (rng.standard_normal(G * K2) * 0.1).astype(np.float32)
    rvar = rng.uniform(0.5, 1.5, G * K2).astype(np.float32)

    nc = _build_program()
    in_maps = _host_inputs(x, conv_w, gamma, beta, rmean, rvar)
    sim = CoreSim(nc)
    for kk, v in in_maps[0].items():
        sim.tensor(kk)[:] = v
    sim.simulate(check_with_hw=False)
    ysim = np.asarray(sim.tensor("y")).astype(np.float32).reshape(
        2, C, ROWS_SUB, WO
    )

    # numpy reference for core 0 region (image 0, output rows 0..32)
    scale = gamma / np.sqrt(rvar + EPS)
    shift = beta - rmean * scale
    xpad = np.pad(x[0], ((0, 0), (1, 1), (1, 1)), mode="reflect")
    sig = np.zeros((G * K2, 32, WO), np.float32)
    for o in range(G * K2):
        for dy in range(K):
            for dx in range(K):
                sig[o] += np.einsum(
                    "crw->rw",
                    conv_w[o, :, dy, dx][:, None, None]
                    * xpad[:, dy : dy + 64 : 2, dx : dx + 128 : 2],
                )
    sig = sig * scale[:, None, None] + shift[:, None, None]
    e = np.exp(sig)
    r = 1.0 / e.sum(0)
    acc = np.zeros((C, 32, WO), np.float32)
    for g in range(G):
        for k in range(K2):
            dy, dx = k // K, k % K
            acc[32 * g : 32 * g + 32] += (
                xpad[32 * g : 32 * g + 32, dy : dy + 64 : 2, dx : dx + 128 : 2]
                * e[g * K2 + k][None]
            )
    ref = (xpad[:, 1:65:2, 1:129:2] - acc * r[None]).astype(np.float32)

    got = np.concatenate([ysim[0], ysim[1]], axis=1)
    err = np.abs(got - ref).max() / np.abs(ref).max()
    print("sim rel err:", err)
